# revision 1
# baseline (speedup 1.0000x reference)
"""Trainium2 Bass kernel for a fused multi-head attention block.

Reference computation (B=2, S=2048, H=1024, NH=16, HD=64):
    qh/kh/vh = (x @ W + b) per head
    energy   = qh @ kh^T  (full S x S per head)
    attn     = softmax(where(mask==0, -1e9, energy) / sqrt(H))
    out      = attn @ vh
    y        = out @ Wfc + bfc + q (residual)
    return LayerNorm(y) * gamma + beta

Sharding: data-parallel over batch (2 groups of 4 cores) x tensor-parallel
over heads (4 heads per core). Wq/Wk/Wv column-sharded, Wfc row-sharded,
ReduceScatter(add) over each 4-core group after fc, then per-core
residual+LayerNorm on its 512-row output slice.

Per-core kernel layout choices:
  * q/k projections produce TRANSPOSED activations qh^T/kh^T [256, S] so the
    scores matmul (contraction over head dim) can run directly from SBUF.
  * scores are computed transposed: energy^T[k, q] tiles, so exp can be
    applied on the PSUM tile and the softmax denominator comes for free via
    an appended ones-column in the attn@V stationary operand (vext).
  * the masked softmax is multiplicative: P = exp(energy/32) * maskT, which
    is exactly equivalent to the reference's -1e9 additive mask since
    exp(-1e9/32) == 0 in fp32.
  * attn@V accumulates out^T[d|1, q] in PSUM over k-tiles; row 64 holds the
    denominator. The divide is done as reciprocal + broadcast-multiply
    (broadcast via a stride-0 DRAM DMA).
"""

import numpy as np
import ml_dtypes

import concourse.bass as bass
import concourse.mybir as mybir
from concourse import bacc, tile
from concourse.bass_utils import run_bass_kernel_spmd

B, S, H, NH = 2, 2048, 1024, 16
HD = H // NH                  # 64
NCORES = 8
TPG = 4                       # cores per tensor-parallel group
HPC = NH // TPG               # 4 heads per core
DC = HPC * HD                 # 256 head-dims per core
SR = S // TPG                 # 512 output rows per core
INV_SCALE = 1.0 / float(H) ** 0.5   # 1/32
EPS = 1e-5

FP = mybir.dt.float32
BF = mybir.dt.bfloat16
F32 = np.float32
BF16 = ml_dtypes.bfloat16

KT = H // 128                 # 8 contraction tiles for projections
ST = S // 128                 # 16 seq tiles
QC = S // 512                 # 4 q-chunks of 512
RT = SR // 128                # 4 row tiles in the final phase
E1 = HD + 1                   # 65: head dims + ones column

ts = bass.ts
AF = mybir.ActivationFunctionType
ALU = mybir.AluOpType


def _build_nc():
    nc = bacc.Bacc(
        "TRN2",
        target_bir_lowering=False,
        debug=False,
        num_devices=NCORES,
    )

    # ---- per-core DRAM I/O ----
    qT = nc.dram_tensor("qT", [H, S], BF, kind="ExternalInput")
    kTt = nc.dram_tensor("kTt", [H, S], BF, kind="ExternalInput")
    vT = nc.dram_tensor("vT", [H, S], BF, kind="ExternalInput")
    maskT = nc.dram_tensor("maskT", [S, S], BF, kind="ExternalInput")
    wq = nc.dram_tensor("wq", [H, DC], BF, kind="ExternalInput")
    wk = nc.dram_tensor("wk", [H, DC], BF, kind="ExternalInput")
    wv = nc.dram_tensor("wv", [H, DC], BF, kind="ExternalInput")
    wfc = nc.dram_tensor("wfc", [DC, H], BF, kind="ExternalInput")
    bq = nc.dram_tensor("bq", [DC, 1], FP, kind="ExternalInput")
    bk = nc.dram_tensor("bk", [DC, 1], FP, kind="ExternalInput")
    bv = nc.dram_tensor("bv", [1, DC], FP, kind="ExternalInput")
    resid = nc.dram_tensor("resid", [SR, H], FP, kind="ExternalInput")
    gamma = nc.dram_tensor("gamma", [1, H], FP, kind="ExternalInput")
    beta = nc.dram_tensor("beta", [1, H], FP, kind="ExternalInput")
    out = nc.dram_tensor("out", [SR, H], FP, kind="ExternalOutput")

    with tile.TileContext(nc) as tc:
        with (
            tc.tile_pool(name="const", bufs=1) as cpool,
            tc.tile_pool(name="stream", bufs=2) as spool,
            tc.tile_pool(name="mask", bufs=4) as mpool,
            tc.tile_pool(name="work", bufs=4) as wpool,
            tc.tile_pool(name="epi", bufs=2) as epool,
            tc.tile_pool(name="fin", bufs=2) as fpool,
            tc.tile_pool(name="psum", bufs=1, space="PSUM") as ppool,
            tc.tile_pool(name="psA", bufs=2, space="PSUM") as ppoolA,
            tc.tile_pool(name="dram", bufs=1, space="DRAM") as dpool,
            tc.tile_pool(name="dram2", bufs=2, space="DRAM") as dpool2,
        ):
            # ---------- constants ----------
            # weights are loaded lazily right before the phase that uses them
            # so the first projection's inputs aren't queued behind them
            w_dram = {"wq": wq, "wk": wk, "wv": wv}
            w_sb = {}

            def load_w(name):
                tiles = []
                for kt in range(KT):
                    t = cpool.tile(
                        [128, DC], BF, tag=f"{name}{kt}", name=f"{name}{kt}"
                    )
                    nc.sync.dma_start(out=t[:], in_=w_dram[name][ts(kt, 128), :])
                    tiles.append(t)
                w_sb[name] = tiles

            load_w("wq")
            bias_sb = {}
            for name, dram in (("bq", bq), ("bk", bk)):
                tiles = []
                for nt in range(2):
                    t = cpool.tile([128, 1], FP, tag=f"{name}{nt}")
                    nc.sync.dma_start(out=t[:], in_=dram[ts(nt, 128), :])
                    tiles.append(t)
                bias_sb[name] = tiles
            # broadcast rows loaded via stride-0 DRAM reads
            bvB = cpool.tile([128, DC], FP, tag="bvB")
            nc.sync.dma_start(out=bvB[:], in_=bv[:].broadcast_to([128, DC]))
            gb_dram = dpool.tile([2, H], BF, tag="gb_dram", name="gb_dram")
            gb_row = fpool.tile([2, H], FP, tag="zt", name="gb_row")
            nc.sync.dma_start(out=gb_row[0:1, :], in_=gamma[:])
            nc.sync.dma_start(out=gb_row[1:2, :], in_=beta[:])
            gb_bf = fpool.tile([2, H], BF, tag="yb", name="gb_bf", bufs=4)
            nc.vector.tensor_copy(gb_bf[:], gb_row[:])
            nc.sync.dma_start(out=gb_dram[:], in_=gb_bf[:])
            gammaB = cpool.tile([128, H], BF, tag="gammaB")
            nc.sync.dma_start(
                out=gammaB[:], in_=gb_dram[0:1, :].broadcast_to([128, H])
            )
            betaB = cpool.tile([128, H], BF, tag="betaB")
            nc.sync.dma_start(
                out=betaB[:], in_=gb_dram[1:2, :].broadcast_to([128, H])
            )

            # ---------- q/k projections (transposed outputs [DC, S]) ----------
            qhT_sb = [cpool.tile([128, S], BF, tag=f"qhT{nt}", name=f"qhT{nt}") for nt in range(2)]
            khT_sb = [cpool.tile([128, S], BF, tag=f"khT{nt}", name=f"khT{nt}") for nt in range(2)]
            for bname, xdram, outsb in (("bq", qT, qhT_sb), ("bk", kTt, khT_sb)):
                wname = "wq" if bname == "bq" else "wk"
                if wname not in w_sb:
                    load_w(wname)
                x_tiles = []
                for kt in range(KT):
                    xt = spool.tile([128, S], BF, tag=f"x{kt}")
                    nc.sync.dma_start(out=xt[:], in_=xdram[ts(kt, 128), :])
                    x_tiles.append(xt)
                for nt in range(2):
                    for qc in range(QC):
                        ps = ppoolA.tile([128, 512], FP, tag="A")
                        for kt in range(KT):
                            nc.tensor.matmul(
                                ps[:],
                                lhsT=w_sb[wname][kt][:, ts(nt, 128)],
                                rhs=x_tiles[kt][:, ts(qc, 512)],
                                start=(kt == 0),
                                stop=(kt == KT - 1),
                            )
                        nc.any.tensor_scalar_add(
                            outsb[nt][:, ts(qc, 512)], ps[:], bias_sb[bname][nt][:]
                        )

            # ---------- v projection (natural layout -> vext [S, 4*65]) ----------
            vext_sb = []
            for st in range(ST):
                t = cpool.tile([128, HPC * E1], BF, tag=f"vext{st}")
                vext_sb.append(t)
            load_w("wv")
            vT_tiles = []
            for ht in range(KT):
                xt = spool.tile([128, S], BF, tag=f"x{ht}")
                nc.sync.dma_start(out=xt[:], in_=vT[ts(ht, 128), :])
                vT_tiles.append(xt)
            for st in range(ST):
                ps = ppoolA.tile([128, DC], FP, tag="A")
                for ht in range(KT):
                    nc.tensor.matmul(
                        ps[:],
                        lhsT=vT_tiles[ht][:, ts(st, 128)],
                        rhs=w_sb["wv"][ht][:],
                        start=(ht == 0),
                        stop=(ht == KT - 1),
                    )
                vx = vext_sb[st]
                for h in range(HPC):
                    nc.vector.memset(vx[:, h * E1 + HD : h * E1 + E1], 1.0)
                v3 = vx.rearrange("p (h e) -> p h e", e=E1)[:, :, 0:HD]
                p3 = ps.rearrange("p (h e) -> p h e", e=HD)
                b3 = bvB.rearrange("p (h e) -> p h e", e=HD)
                nc.vector.tensor_add(v3, p3, b3)

            # ---------- attention: head pairs x q-halves ----------
            # Software-pipelined emission: each attn@V is emitted LAG
            # iterations after its scores/exp/mask-mul so the in-order PE
            # stream has independent scores work while the previous block's
            # B accumulator drains through the divide epilogue.
            LAG = 5
            outT_sc = [cpool.tile([128, S], BF, tag=f"oT{nt}", name=f"oT{nt}") for nt in range(2)]
            B_tiles = {}
            pending = []

            def emit_epilogue(hp, half, hh, Bt):
                q0 = 1024 * half
                dn65 = epool.tile([E1, 1024], FP, tag="dn65", name="dn65")
                nc.any.tensor_copy(dn65[64:65, :], Bt[64:65, :])
                dnP = epool.tile([128, 8], FP, tag="dnP", name="dnP")
                nc.sync.dma_start(out=dnP[:], in_=dn65[64:65, :])
                rcP = epool.tile([128, 8], FP, tag="rcP", name="rcP")
                nc.vector.reciprocal(rcP[:], dnP[:])
                rdram = dpool2.tile([1, 1024], FP, tag="rdram", name="rdram")
                nc.sync.dma_start(out=rdram[:], in_=rcP[:])
                rb = epool.tile([64, 1024], FP, tag="rb", name="rb")
                nc.sync.dma_start(out=rb[:], in_=rdram[:].broadcast_to([64, 1024]))
                if hh == 0:
                    nc.vector.tensor_mul(
                        outT_sc[hp][0:64, q0 : q0 + 1024], Bt[0:64, :], rb[:]
                    )
                else:
                    osc = epool.tile([64, 1024], BF, tag="osc", name="osc")
                    nc.vector.tensor_mul(osc[:], Bt[0:64, :], rb[:])
                    nc.sync.dma_start(
                        out=outT_sc[hp][64:128, q0 : q0 + 1024], in_=osc[:]
                    )

            def emit_attnv(ent):
                hp, half, kj, hh, Pm = ent
                h = 2 * hp + hh
                Bt = B_tiles[(hp, half)][hh]
                for c in range(2):
                    nc.tensor.matmul(
                        Bt[:, ts(c, 512)],
                        lhsT=vext_sb[kj][:, h * E1 : (h + 1) * E1],
                        rhs=Pm[:, ts(c, 512)],
                        start=(kj == 0),
                        stop=(kj == ST - 1),
                    )
                if kj == ST - 1:
                    emit_epilogue(hp, half, hh, Bt)

            cur_mask = None
            for hp in range(2):
                for half in range(2):
                    q0 = 1024 * half
                    B_tiles[(hp, half)] = [
                        ppool.tile(
                            [E1, 1024], FP, tag=f"attB{hh}",
                            name=f"attB{hh}_{hp}_{half}",
                        )
                        for hh in range(2)
                    ]
                    for kj in range(ST):
                        mt = mpool.tile([128, 1024], BF, tag="mask", name="mask")
                        nc.sync.dma_start(
                            out=mt[:], in_=maskT[ts(kj, 128), q0 : q0 + 1024]
                        )
                        for hh in range(2):
                            hb = 64 * hh
                            A = ppoolA.tile([128, 1024], FP, tag="A", name="A")
                            for c in range(2):
                                nc.tensor.matmul(
                                    A[:, ts(c, 512)],
                                    lhsT=khT_sb[hp][hb : hb + 64, ts(kj, 128)],
                                    rhs=qhT_sb[hp][hb : hb + 64, q0 + 512 * c : q0 + 512 * (c + 1)],
                                    start=True,
                                    stop=True,
                                )
                            P = wpool.tile([128, 1024], BF, tag="P", name="P", bufs=3)
                            nc.scalar.activation(P[:], A[:], AF.Exp, scale=INV_SCALE)
                            Pm = wpool.tile(
                                [128, 1024], BF, tag="Pm", name="Pm", bufs=7
                            )
                            nc.vector.tensor_mul(Pm[:], P[:], mt[:])
                            pending.append((hp, half, kj, hh, Pm))
                            if len(pending) > LAG:
                                emit_attnv(pending.pop(0))
            for ent in pending:
                emit_attnv(ent)

            wfc_sb = []
            for dg in range(2):
                t = cpool.tile([128, H], BF, tag=f"wfc{dg}", name=f"wfc{dg}")
                nc.sync.dma_start(out=t[:], in_=wfc[ts(dg, 128), :])
                wfc_sb.append(t)

            # ---------- fc partial + chunked reduce-scatter ----------
            # y_part chunk i covers s-rows [512i, 512(i+1)); after each chunk's
            # four s-tiles finish, a ReduceScatter over the 4-core group hands
            # this core rows [512i+128r, 512i+128(r+1)) (r = group rank).
            y_chunks = [
                dpool.tile([SR, H], BF, tag=f"y_part{i}", name=f"y_part{i}")
                for i in range(RT)
            ]
            z_chunks = [
                dpool.tile([128, H], BF, tag=f"z{i}", name=f"z{i}")
                for i in range(RT)
            ]
            for st in range(ST):
                ps = ppoolA.tile([128, H], FP, tag="A")
                for dg in range(2):
                    for hc in range(2):
                        nc.tensor.matmul(
                            ps[:, ts(hc, 512)],
                            lhsT=outT_sc[dg][:, ts(st, 128)],
                            rhs=wfc_sb[dg][:, ts(hc, 512)],
                            start=(dg == 0),
                            stop=(dg == 1),
                        )
                yb = fpool.tile([128, H], BF, tag="yb", bufs=4)
                nc.any.tensor_copy(yb[:], ps[:])
                nc.sync.dma_start(
                    out=y_chunks[st // 4][ts(st % 4, 128), :], in_=yb[:]
                )
                if st % 4 == 3:
                    nc.gpsimd.collective_compute(
                        "ReduceScatter",
                        ALU.add,
                        replica_groups=[[0, 1, 2, 3], [4, 5, 6, 7]],
                        ins=[y_chunks[st // 4][:]],
                        outs=[z_chunks[st // 4][:]],
                    )

            # ---------- residual + layernorm on own 4x128-row slices ----------
            for rt in range(RT):
                zbf = fpool.tile([128, H], BF, tag="zbf", bufs=4)
                nc.sync.dma_start(out=zbf[:], in_=z_chunks[rt][:])
                rs = fpool.tile([128, H], FP, tag="rs")
                nc.sync.dma_start(out=rs[:], in_=resid[ts(rt, 128), :])
                musum = fpool.tile([128, 1], FP, tag="musum")
                zt = fpool.tile([128, H], FP, tag="zt")
                # y = z + resid, accumulating row sums for the mean
                nc.vector.scalar_tensor_tensor(
                    zt[:], zbf[:], 0.0, rs[:], ALU.add, ALU.add, accum_out=musum[:]
                )
                nmu = fpool.tile([128, 1], FP, tag="nmu")
                nc.vector.tensor_scalar_mul(nmu[:], musum[:], -1.0 / H)
                nc.scalar.activation(zt[:], zt[:], AF.Identity, bias=nmu[:])
                ssq = fpool.tile([128, 1], FP, tag="ssq")
                nc.vector.scalar_tensor_tensor(
                    rs[:], zt[:], 0.0, zt[:], ALU.add, ALU.mult, accum_out=ssq[:]
                )
                varp = fpool.tile([128, 1], FP, tag="varp")
                nc.vector.tensor_scalar(
                    varp[:], ssq[:], 1.0 / H, EPS, ALU.mult, ALU.add
                )
                sdev = fpool.tile([128, 1], FP, tag="sdev")
                nc.scalar.activation(sdev[:], varp[:], AF.Sqrt)
                rstd = fpool.tile([128, 1], FP, tag="rstd")
                nc.vector.reciprocal(rstd[:], sdev[:])
                nc.vector.scalar_tensor_tensor(
                    rs[:], zt[:], rstd[:], gammaB[:], ALU.mult, ALU.mult
                )
                ot = fpool.tile([128, H], FP, tag="ot")
                nc.vector.tensor_add(ot[:], rs[:], betaB[:])
                nc.sync.dma_start(out=out[ts(rt, 128), :], in_=ot[:])

    nc.compile()
    return nc


_NC_CACHE = {}


def _get_nc():
    if "nc" not in _NC_CACHE:
        _NC_CACHE["nc"] = _build_nc()
    return _NC_CACHE["nc"]


def _prep_inputs(q, k, v, mask, Wq, bq, Wk, bk, Wv, bv, Wfc, bfc, gamma, beta):
    """Build the 8 per-core input maps on the host (sharding + layout)."""
    q = np.asarray(q, F32)
    k = np.asarray(k, F32)
    v = np.asarray(v, F32)
    mask = np.asarray(mask)
    in_maps = []
    qT_b, kT_b, vT_b, maskT_b = [], [], [], []
    for b in range(B):
        qT_b.append(np.ascontiguousarray(q[b].T).astype(BF16))
        kT_b.append(np.ascontiguousarray(k[b].T).astype(BF16))
        vT_b.append(np.ascontiguousarray(v[b].T).astype(BF16))
        maskT_b.append(np.ascontiguousarray(mask[b, 0].T).astype(BF16))
    Wq_bf, Wk_bf, Wv_bf, Wfc_bf = (
        np.asarray(w, F32).astype(BF16) for w in (Wq, Wk, Wv, Wfc)
    )
    for c in range(NCORES):
        b, g = c // TPG, c % TPG
        cols = slice(g * DC, (g + 1) * DC)
        in_maps.append({
            "qT": qT_b[b],
            "kTt": kT_b[b],
            "vT": vT_b[b],
            "maskT": maskT_b[b],
            "wq": np.ascontiguousarray(Wq_bf[:, cols]),
            "wk": np.ascontiguousarray(Wk_bf[:, cols]),
            "wv": np.ascontiguousarray(Wv_bf[:, cols]),
            "wfc": np.ascontiguousarray(Wfc_bf[cols, :]),
            "bq": np.asarray(bq, F32)[cols].reshape(DC, 1),
            "bk": np.asarray(bk, F32)[cols].reshape(DC, 1),
            "bv": np.asarray(bv, F32)[cols].reshape(1, DC),
            "resid": np.ascontiguousarray(
                np.concatenate(
                    [
                        q[b, 512 * i + 128 * g : 512 * i + 128 * (g + 1)]
                        for i in range(RT)
                    ]
                )
                + np.asarray(bfc, F32)[None, :]
            ),
            "gamma": np.asarray(gamma, F32).reshape(1, H),
            "beta": np.asarray(beta, F32).reshape(1, H),
        })
    return in_maps


_LAST_RUN_S = [None]


def kernel(q, k, v, mask, Wq, bq, Wk, bk, Wv, bv, Wfc, bfc, gamma, beta):
    import time

    nc = _get_nc()
    in_maps = _prep_inputs(
        q, k, v, mask, Wq, bq, Wk, bk, Wv, bv, Wfc, bfc, gamma, beta
    )
    t0 = time.perf_counter()
    res = run_bass_kernel_spmd(nc, in_maps, list(range(NCORES)))
    _LAST_RUN_S[0] = time.perf_counter() - t0
    full = np.empty((B, S, H), F32)
    for c in range(NCORES):
        b, r = c // TPG, c % TPG
        o = res.results[c]["out"]
        for i in range(RT):
            full[b, 512 * i + 128 * r : 512 * i + 128 * (r + 1)] = o[
                128 * i : 128 * (i + 1)
            ]
    return full



# revision 38
# speedup vs baseline: 1.4153x; 1.4153x over previous
"""Trainium2 Bass kernel for a fused multi-head attention block.

Reference computation (B=2, S=2048, H=1024, NH=16, HD=64):
    qh/kh/vh = (x @ W + b) per head
    energy   = qh @ kh^T  (full S x S per head)
    attn     = softmax(where(mask==0, -1e9, energy) / sqrt(H))
    out      = attn @ vh
    y        = out @ Wfc + bfc + q (residual)
    return LayerNorm(y) * gamma + beta

Sharding: data-parallel over batch (2 groups of 4 cores) x tensor-parallel
over heads (4 heads per core). Wq/Wk/Wv column-sharded, Wfc row-sharded,
ReduceScatter(add) over each 4-core group after fc, then per-core
residual+LayerNorm on its 512-row output slice.

Per-core kernel design (v3):
  * All four weight matmuls (q/k/v projections and fc) run in fp8e4m3 with
    MatmulPerfMode.DoubleRow: operands are packed host-side as pairs of
    128-row contraction tiles side by side in the free dim, so each
    DoubleRow matmul contracts 256 rows at 0.5 cycles/output-row (4x the
    bf16 rate). Weights are scaled by 64 on the host so their ~0.02-sigma
    values stay clear of the fp8 subnormal range; epilogues rescale by
    1/64.
  * Scores / attn@V stay bf16: the attention inner loop is balanced against
    the Activation engine's exp wall (1024 free-elems per iteration on
    both), which keeps the PE continuously busy (no pstate drops).
  * The modeled HWDGE descriptor generator serializes at ~650ns per DMA
    regardless of size, so bulk inputs ride few, large DMAs on the SP
    queue, emitted in consumption order; small/latency-tolerant transfers
    (biases, epilogue round-trips, LN I/O) ride the Pool engine's separate
    SWDGE path.
  * Projection and v-projection compute is emitted *inside* the attention
    loop at points chosen so their input DMAs have landed and the PE
    stream never waits: q/k first halves before attention, k second half
    at kj==5, v-projection in two 8-tile chunks at kj==1/kj==8, q second
    half at the start of the second head-pair block.
  * The masked softmax is multiplicative: P = exp(energy/32) * maskT, with
    mask tiles cached in SBUF and shared by both head pairs of a q-half.
  * attn@V accumulates out^T[d|1, q] in PSUM over k-tiles (ones-column in
    vext gives the softmax denominator); divide = reciprocal + broadcast
    multiply fused with the x64 rescale, written fp8 for the DoubleRow fc.
  * fc for the first q-half is deferred into the second half's attention
    stream (its epilogue chains finish in the background); ReduceScatter
    is chunked; all residual+LayerNorm chains run at the end, spread
    across Act (bias/scale ops), DVE (accumulating ops), and Pool
    (plain elementwise), so the four chains pipeline.
"""

import numpy as np
import ml_dtypes

import concourse.bass as bass
import concourse.mybir as mybir
from concourse import bacc, tile
from concourse.bass_utils import run_bass_kernel_spmd

B, S, H, NH = 2, 2048, 1024, 16
HD = H // NH                  # 64
NCORES = 8
TPG = 4                       # cores per tensor-parallel group
HPC = NH // TPG               # 4 heads per core
DC = HPC * HD                 # 256 head-dims per core
SR = S // TPG                 # 512 output rows per core
INV_SCALE = 1.0 / float(H) ** 0.5   # 1/32
EPS = 1e-5
WS = 64.0                     # fp8 weight prescale
IWS = 1.0 / WS
IWS2 = 1.0 / (WS * WS)

FP = mybir.dt.float32
BF = mybir.dt.bfloat16
F8 = mybir.dt.float8e4
F32 = np.float32
BF16 = ml_dtypes.bfloat16
FP8 = ml_dtypes.float8_e4m3

KP = H // 256                 # 4 fp8 pair-tiles over the contraction dim
ST = S // 128                 # 16 seq tiles
QC = S // 512                 # 4 q-chunks of 512
RT = SR // 128                # 4 row tiles in the final phase
E1 = HD + 1                   # 65: head dims + ones column

ts = bass.ts
AF = mybir.ActivationFunctionType
ALU = mybir.AluOpType
DR = mybir.MatmulPerfMode.DoubleRow


def _build_nc():
    nc = bacc.Bacc(
        "TRN2",
        target_bir_lowering=False,
        debug=False,
        num_devices=NCORES,
    )

    # ---- per-core DRAM I/O ----
    # activation pair layout: [512, 2S]; row p of pair-tile jp holds
    # contraction rows 256*jp+p (cols [0,S)) and 256*jp+128+p (cols [S,2S)).
    # weight pack: [128, 8*DC]; col jp*2*DC + i*DC + d = W[256*jp+128*i+p, d].
    qp = nc.dram_tensor("qp", [512, 2 * S], F8, kind="ExternalInput")
    kp = nc.dram_tensor("kp", [512, 2 * S], F8, kind="ExternalInput")
    vp = nc.dram_tensor("vp", [512, 2 * S], F8, kind="ExternalInput")
    maskT = nc.dram_tensor("maskT", [S, S], BF, kind="ExternalInput")
    wqp = nc.dram_tensor("wqp", [128, 2 * H], F8, kind="ExternalInput")
    wkp = nc.dram_tensor("wkp", [128, 2 * H], F8, kind="ExternalInput")
    wvp = nc.dram_tensor("wvp", [128, 2 * H], F8, kind="ExternalInput")
    wfcp = nc.dram_tensor("wfcp", [128, 2 * H], F8, kind="ExternalInput")
    bqk = nc.dram_tensor("bqk", [128, 4], FP, kind="ExternalInput")
    bv = nc.dram_tensor("bv", [1, DC], FP, kind="ExternalInput")
    resid = nc.dram_tensor("resid", [SR, H], BF, kind="ExternalInput")
    gamma = nc.dram_tensor("gamma", [1, H], FP, kind="ExternalInput")
    beta = nc.dram_tensor("beta", [1, H], FP, kind="ExternalInput")
    out = nc.dram_tensor("out", [SR, H], BF, kind="ExternalOutput")

    def pairs(t):
        # [128, 2X] region -> [128, 2, X] AP
        return t.rearrange("p (two x) -> p two x", two=2)

    with tile.TileContext(nc) as tc:
        with (
            tc.tile_pool(name="const", bufs=1) as cpool,
            tc.tile_pool(name="mask", bufs=1) as mpool,
            tc.tile_pool(name="work", bufs=4) as wpool,
            tc.tile_pool(name="epi", bufs=2) as epool,
            tc.tile_pool(name="fin", bufs=2) as fpool,
            tc.tile_pool(name="psum", bufs=1, space="PSUM") as ppool,
            tc.tile_pool(name="psA", bufs=2, space="PSUM") as ppoolA,
            tc.tile_pool(name="dram", bufs=1, space="DRAM") as dpool,
            tc.tile_pool(name="dram2", bufs=2, space="DRAM") as dpool2,
        ):
            # ---------- bulk SP-queue DMAs: weights, then q/k first waves ----
            w_sb = {}
            for name, dram in (
                ("wqp", wqp), ("wkp", wkp), ("wvp", wvp), ("wfc", wfcp)
            ):
                t = cpool.tile([128, 2 * H], F8, tag=name, name=name)
                nc.sync.dma_start(out=t[:], in_=dram[:])
                w_sb[name] = t

            def wslice(name, jp):
                # weight pair-block jp as a [128, 2, DC] AP
                return pairs(w_sb[name][:, jp * 2 * DC : (jp + 1) * 2 * DC])

            x_sb = {}
            for xn in ("q", "k", "v"):
                x_sb[xn] = [
                    cpool.tile([128, 2 * S], F8, tag=f"x{xn}{jp}", name=f"x{xn}{jp}")
                    for jp in range(KP)
                ]

            def emit_xwave(xn, dram, w):
                # one 1MB wave: q-columns [1024w, 1024w+1024) of each pair tile
                for jp in range(KP):
                    nc.sync.dma_start(
                        out=pairs(x_sb[xn][jp])[:, :, ts(w, 1024)],
                        in_=pairs(dram[ts(jp, 128), :])[:, :, ts(w, 1024)],
                    )

            emit_xwave("q", qp, 0)
            emit_xwave("k", kp, 0)
            emit_xwave("v", vp, 0)

            # ---------- small constants via the Pool SWDGE path ----------
            bqk_sb = cpool.tile([128, 4], FP, tag="bqk", name="bqk")
            nc.gpsimd.dma_start(out=bqk_sb[:], in_=bqk[:])
            bias_col = {"bq": (0, 1), "bk": (2, 3)}
            bvB = cpool.tile([128, DC], FP, tag="bvB")
            nc.gpsimd.dma_start(out=bvB[:], in_=bv[:].broadcast_to([128, DC]))
            gb_dram = dpool.tile([2, H], BF, tag="gb_dram", name="gb_dram")
            gb_row = fpool.tile([2, H], FP, tag="zt", name="gb_row")
            nc.gpsimd.dma_start(out=gb_row[0:1, :], in_=gamma[:])
            nc.gpsimd.dma_start(out=gb_row[1:2, :], in_=beta[:])
            gb_bf = fpool.tile([2, H], BF, tag="gbf", name="gb_bf")
            nc.vector.tensor_copy(gb_bf[:], gb_row[:])
            nc.gpsimd.dma_start(out=gb_dram[:], in_=gb_bf[:])
            gammaB = cpool.tile([128, H], BF, tag="gammaB")
            nc.gpsimd.dma_start(
                out=gammaB[:], in_=gb_dram[0:1, :].broadcast_to([128, H])
            )
            betaB = cpool.tile([128, H], BF, tag="betaB")
            nc.gpsimd.dma_start(
                out=betaB[:], in_=gb_dram[1:2, :].broadcast_to([128, H])
            )

            # ---------- q/k projections (transposed outputs [DC, S]) ----------
            qhT_sb = [cpool.tile([128, S], BF, tag=f"qhT{nt}", name=f"qhT{nt}") for nt in range(2)]
            khT_sb = [cpool.tile([128, S], BF, tag=f"khT{nt}", name=f"khT{nt}") for nt in range(2)]
            proj_out = {"q": qhT_sb, "k": khT_sb}
            proj_w = {"q": "wqp", "k": "wkp"}
            proj_b = {"q": "bq", "k": "bk"}

            def emit_proj(xn, qcs, nts=(0, 1)):
                wname = proj_w[xn]
                c0, c1 = bias_col[proj_b[xn]]
                for qc in qcs:
                    for nt in nts:
                        ps = ppoolA.tile([128, 512], FP, tag="A", name="ps")
                        for jp in range(KP):
                            nc.tensor.matmul(
                                ps[:],
                                lhsT=wslice(wname, jp)[:, :, ts(nt, 128)],
                                rhs=pairs(x_sb[xn][jp])[:, :, ts(qc, 512)],
                                start=(jp == 0),
                                stop=(jp == KP - 1),
                                perf_mode=DR,
                            )
                        nc.vector.tensor_scalar(
                            proj_out[xn][nt][:, ts(qc, 512)], ps[:],
                            IWS, bqk_sb[:, c0 + nt : c0 + nt + 1],
                            ALU.mult, ALU.add,
                        )

            emit_proj("q", (0, 1))
            emit_proj("k", (0, 1))

            # ---------- v projection (natural layout -> vext [S, 4*65]) ------
            vext_sb = [
                cpool.tile([128, HPC * E1], BF, tag=f"vext{st}", name=f"vext{st}")
                for st in range(ST)
            ]

            def emit_vproj(sts):
                for st in sts:
                    ps = ppoolA.tile([128, DC], FP, tag="A", name="ps")
                    for jp in range(KP):
                        nc.tensor.matmul(
                            ps[:],
                            lhsT=pairs(x_sb["v"][jp])[:, :, ts(st, 128)],
                            rhs=wslice("wvp", jp)[:],
                            start=(jp == 0),
                            stop=(jp == KP - 1),
                            perf_mode=DR,
                        )
                    vx = vext_sb[st]
                    for h in range(HPC):
                        nc.gpsimd.memset(vx[:, h * E1 + HD : h * E1 + E1], 1.0)
                    v3 = vx.rearrange("p (h e) -> p h e", e=E1)[:, :, 0:HD]
                    p3 = ps.rearrange("p (h e) -> p h e", e=HD)
                    b3 = bvB.rearrange("p (h e) -> p h e", e=HD)
                    nc.vector.scalar_tensor_tensor(
                        v3, p3, IWS, b3, ALU.mult, ALU.add
                    )

            ones_sb = cpool.tile([E1, HD], BF, tag="ones", name="ones_sb")
            nc.gpsimd.memset(ones_sb[64:65, :], 1.0)

            # outT in fp8 (x64) column-split: head h -> rows 64*(h%2),
            # cols S*(h//2) + q. fc DoubleRow pairs the two column groups.
            outT_sc = cpool.tile([128, 2 * S], F8, tag="oT", name="oT")

            # ---------- fc partial + chunked reduce-scatter ----------
            y_chunks = [
                dpool.tile([SR, H], BF, tag=f"y_part{i}", name=f"y_part{i}")
                for i in range(RT)
            ]
            z_chunks = [
                dpool.tile([128, H], BF, tag=f"z{i}", name=f"z{i}")
                for i in range(RT)
            ]

            def emit_fc(st, on_act=False):
                ps = ppoolA.tile([128, H], FP, tag="A", name="ps")
                for hc in range(2):
                    nc.tensor.matmul(
                        ps[:, ts(hc, 512)],
                        lhsT=pairs(outT_sc)[:, :, ts(st, 128)],
                        rhs=pairs(w_sb["wfc"])[:, :, ts(hc, 512)],
                        start=True,
                        stop=True,
                        perf_mode=DR,
                    )
                yb = fpool.tile([128, H], BF, tag="yb", bufs=4)
                # PSUM->SBUF copy: DVE mid-attention (Act is the wall);
                # alternate DVE/Act for the tail chunks where Act is free
                if on_act:
                    nc.scalar.activation(yb[:], ps[:], AF.Identity)
                else:
                    nc.vector.tensor_copy(yb[:], ps[:])
                nc.sync.dma_start(
                    out=y_chunks[st // 4][ts(st % 4, 128), :], in_=yb[:]
                )
                if st % 4 == 3:
                    nc.gpsimd.collective_compute(
                        "ReduceScatter",
                        ALU.add,
                        replica_groups=[[0, 1, 2, 3], [4, 5, 6, 7]],
                        ins=[y_chunks[st // 4][:]],
                        outs=[z_chunks[st // 4][:]],
                    )
                    emit_ln_load(st // 4)

            ln_in = {}

            def emit_ln_load(rt):
                # z + residual loads, emitted right after the RS so the data
                # is resident when the LN chain runs at the tail
                zf = fpool.tile([128, H], BF, tag="zf", bufs=2)
                nc.sync.dma_start(out=zf[:], in_=z_chunks[rt][:])
                rs = fpool.tile([128, H], BF, tag="rs")
                nc.sync.dma_start(out=rs[:], in_=resid[ts(rt, 128), :])
                ln_in[rt] = (zf, rs)

            def emit_ln(rt):
                # residual + layernorm on own 128-row slice, emitted at the
                # end with all RS chunks in flight so the four chains
                # pipeline across Act/DVE/Pool. y = z/WS^2 + resid.
                zf, rs = ln_in[rt]
                musum = fpool.tile([128, 1], FP, tag="musum")
                zt = fpool.tile([128, H], FP, tag="zt")
                nc.vector.scalar_tensor_tensor(
                    zt[:], zf[:], IWS2, rs[:], ALU.mult, ALU.add,
                    accum_out=musum[:],
                )
                nmu = fpool.tile([128, 1], FP, tag="nmu")
                nc.vector.tensor_scalar_mul(nmu[:], musum[:], -1.0 / H)
                ssq = fpool.tile([128, 1], FP, tag="ssq")
                nc.scalar.activation(
                    rs[:], zt[:], AF.Square, bias=nmu[:], accum_out=ssq[:]
                )
                varp = fpool.tile([128, 1], FP, tag="varp")
                nc.vector.tensor_scalar(
                    varp[:], ssq[:], 1.0 / H, EPS, ALU.mult, ALU.add
                )
                sdev = fpool.tile([128, 1], FP, tag="sdev")
                nc.scalar.activation(sdev[:], varp[:], AF.Sqrt)
                rstd = fpool.tile([128, 1], FP, tag="rstd")
                nc.vector.reciprocal(rstd[:], sdev[:])
                # zn = (zt + nmu) * rstd in bf16, then gamma/beta in bf16
                # (DVE gets its 2x mode; the fp32 conversion happens on the
                # host after the gather). ~0.2% output quantization, well
                # inside the 2e-2 budget.
                zn = fpool.tile([128, H], BF, tag="zn", bufs=2)
                with nc.allow_low_precision(
                    reason="bf16 LN outputs: 0.4%-of-element error on the "
                    "final normalized values, budget is 2e-2"
                ):
                    nc.scalar.activation(
                        zn[:], zt[:], AF.Identity, bias=nmu[:], scale=rstd[:]
                    )
                    zg = fpool.tile([128, H], BF, tag="zg", bufs=2)
                    nc.vector.tensor_mul(zg[:], zn[:], gammaB[:])
                    ob = fpool.tile([128, H], BF, tag="ob", bufs=2)
                    nc.vector.tensor_add(ob[:], zg[:], betaB[:])
                nc.sync.dma_start(out=out[ts(rt, 128), :], in_=ob[:])

            # ---------- attention: q-halves x head pairs ----------
            # Software-pipelined emission: each attn@V is emitted LAG
            # iterations after its scores/exp/mask-mul so the in-order PE
            # stream has independent scores work while the previous block's
            # B accumulator drains through the divide epilogue.
            LAG = 5
            B_tiles = {}
            pending = []

            def emit_epilogue(hp, half, hh, Bt, tail=False):
                q0 = 1024 * half
                # reciprocal of the ones-row denominator, broadcast over the
                # 64 head dims. Mid-kernel epilogues are fully hidden under
                # the attention stream, so they use a DRAM-round-trip
                # broadcast on the Pool SWDGE path (no PSUM contention).
                # The last block's epilogues sit on the critical path into
                # the tail fc, so they use a ~3us PE ones-matmul broadcast
                # instead (the PSUM A rotation is idle by then).
                dn = epool.tile([E1, 1024], BF, tag="dn", name="dn")
                with nc.allow_low_precision(
                    reason="bf16 softmax denominators: 0.4% on a 50x-"
                    "diluted signal, far inside the 2e-2 budget"
                ):
                    nc.vector.reciprocal(dn[64:65, :], Bt[64:65, :])
                rb = epool.tile([64, 1024], BF, tag="rb", name="rb")
                if tail:
                    rbp = ppoolA.tile([64, 1024], FP, tag="A", name="rbp")
                    for c in range(2):
                        nc.tensor.matmul(
                            rbp[:, ts(c, 512)],
                            lhsT=ones_sb[64:65, :],
                            rhs=dn[64:65, ts(c, 512)],
                            start=True,
                            stop=True,
                        )
                    # DVE ops may read only one PSUM operand; stage the
                    # broadcast in SBUF (on Act, which is idle at the tail)
                    nc.scalar.activation(rb[:], rbp[:], AF.Identity)
                else:
                    rdram = dpool2.tile(
                        [1, 1024], BF, tag="rdram", name="rdram"
                    )
                    nc.gpsimd.dma_start(out=rdram[:], in_=dn[64:65, :])
                    nc.gpsimd.dma_start(
                        out=rb[:], in_=rdram[:].broadcast_to([64, 1024])
                    )
                if hh == 0:
                    nc.vector.scalar_tensor_tensor(
                        outT_sc[0:64, S * hp + q0 : S * hp + q0 + 1024],
                        Bt[0:64, :], WS, rb[:], ALU.mult, ALU.mult,
                    )
                else:
                    osc = epool.tile([64, 1024], F8, tag="osc", name="osc")
                    nc.vector.scalar_tensor_tensor(
                        osc[:], Bt[0:64, :], WS, rb[:], ALU.mult, ALU.mult
                    )
                    nc.gpsimd.dma_start(
                        out=outT_sc[64:128, S * hp + q0 : S * hp + q0 + 1024],
                        in_=osc[:],
                    )

            def emit_attnv(ent):
                hp, half, kj, hh, Pm = ent
                h = 2 * hp + hh
                Bt = B_tiles[(hp, half)][hh]
                for c in range(2):
                    nc.tensor.matmul(
                        Bt[:, ts(c, 512)],
                        lhsT=vext_sb[kj][:, h * E1 : (h + 1) * E1],
                        rhs=Pm[:, ts(c, 512)],
                        start=(kj == 0),
                        stop=(kj == ST - 1),
                    )
                if kj == ST - 1:
                    emit_epilogue(
                        hp, half, hh, Bt, tail=(half == 1 and hp == 1)
                    )

            # deferred work hooks, emitted into the PE / SP-DMA streams at
            # chosen attention-loop positions: late enough that their input
            # DMAs have landed, fine-grained (<=~850ns of PE work per hook)
            # so the 2-deep PSUM A rotation keeps the exp pipeline fed.
            pe_hooks = {}

            def add_hook(key, fn):
                pe_hooks.setdefault(key, []).append(fn)

            for i in range(8):  # vproj st0..7, one tile (~430ns) per hook
                add_hook(
                    (0, 0, 1 + i // 2, i % 2),
                    lambda s=i: emit_vproj((s,)),
                )
            for i in range(4):  # k second half, one (qc,nt) per hook
                add_hook(
                    (0, 0, 5 + i, 0),
                    lambda qc=2 + i // 2, nt=i % 2: emit_proj(
                        "k", (qc,), (nt,)
                    ),
                )
            for i in range(8):  # vproj st8..15
                add_hook(
                    (0, 0, 9 + i // 2, i % 2),
                    lambda s=8 + i: emit_vproj((s,)),
                )
            for i in range(4):  # q second half, every other kj
                add_hook(
                    (0, 1, 2 * i, 0),
                    lambda qc=2 + i // 2, nt=i % 2: emit_proj(
                        "q", (qc,), (nt,)
                    ),
                )
            for i in range(8):  # first half's fc, after its epilogues drain
                add_hook((1, 0, 4 + i, 0), lambda s=i: emit_fc(s))
            dma_hooks = {
                (0, 0, 3): lambda: emit_xwave("k", kp, 1),
                (0, 0, 5): lambda: emit_xwave("v", vp, 1),
                (0, 0, 10): lambda: emit_xwave("q", qp, 1),
            }

            # mask tiles cached per half, shared across both head pairs
            mask_tiles = [
                mpool.tile([128, 1024], BF, tag=f"mask{kj}", name=f"mask{kj}")
                for kj in range(ST)
            ]
            for half in range(2):
                q0 = 1024 * half
                for hp in range(2):
                    B_tiles[(hp, half)] = [
                        ppool.tile(
                            [E1, 1024], FP, tag=f"attB{hh}",
                            name=f"attB{hh}_{hp}_{half}",
                        )
                        for hh in range(2)
                    ]
                    for kj in range(ST):
                        mt = mask_tiles[kj]
                        if hp == 0:
                            nc.sync.dma_start(
                                out=mt[:],
                                in_=maskT[ts(kj, 128), q0 : q0 + 1024],
                            )
                            hook = dma_hooks.pop((half, hp, kj), None)
                            if hook:
                                hook()
                        for hh in range(2):
                            hb = 64 * hh
                            A = ppoolA.tile([128, 1024], FP, tag="A", name="A")
                            for c in range(2):
                                nc.tensor.matmul(
                                    A[:, ts(c, 512)],
                                    lhsT=khT_sb[hp][hb : hb + 64, ts(kj, 128)],
                                    rhs=qhT_sb[hp][hb : hb + 64, q0 + 512 * c : q0 + 512 * (c + 1)],
                                    start=True,
                                    stop=True,
                                )
                            P = wpool.tile([128, 1024], BF, tag="P", name="P", bufs=5)
                            nc.scalar.activation(P[:], A[:], AF.Exp, scale=INV_SCALE)
                            Pm = wpool.tile(
                                [128, 1024], BF, tag="Pm", name="Pm", bufs=10
                            )
                            nc.vector.tensor_mul(Pm[:], P[:], mt[:])
                            pending.append((half, hp, kj, hh, Pm))
                            for fn in pe_hooks.pop((half, hp, kj, hh), ()):
                                fn()
                            need = 9 if pending[0][2] < 2 else LAG
                            while len(pending) > need:
                                ent = pending.pop(0)
                                emit_attnv(
                                    (ent[1], ent[0], ent[2], ent[3], ent[4])
                                )
                                if pending:
                                    need = (
                                        9 if pending[0][2] < 2 else LAG
                                    )
            # tail: drain the last attn@Vs, then the second half's fc + LN
            for ent in pending:
                emit_attnv((ent[1], ent[0], ent[2], ent[3], ent[4]))
            pending = []
            for st in range(8, 16):
                emit_fc(st, on_act=(st % 2 == 1))
            for rt in range(RT):
                emit_ln(rt)

    nc.compile()
    return nc


_NC_CACHE = {}


def _get_nc():
    if "nc" not in _NC_CACHE:
        _NC_CACHE["nc"] = _build_nc()
    return _NC_CACHE["nc"]


def _pack_x(xT):
    """[1024, X] -> [512, 2X] fp8 activation pair-tile layout."""
    X = xT.shape[1]
    return np.ascontiguousarray(
        xT.reshape(4, 2, 128, X).transpose(0, 2, 1, 3).reshape(512, 2 * X)
    ).astype(FP8)


def _pack_w(w):
    """[1024, DC] (prescaled) -> [128, 8*DC] fp8 weight pack."""
    d = w.shape[1]
    return np.ascontiguousarray(
        w.reshape(4, 2, 128, d).transpose(2, 0, 1, 3).reshape(128, 8 * d)
    ).astype(FP8)


def _prep_inputs(q, k, v, mask, Wq, bq, Wk, bk, Wv, bv, Wfc, bfc, gamma, beta):
    """Build the 8 per-core input maps on the host (sharding + layout)."""
    q = np.asarray(q, F32)
    k = np.asarray(k, F32)
    v = np.asarray(v, F32)
    mask = np.asarray(mask)
    in_maps = []
    qp_b, kp_b, vp_b, maskT_b = [], [], [], []
    for b in range(B):
        qp_b.append(_pack_x(q[b].T))
        kp_b.append(_pack_x(k[b].T))
        vp_b.append(_pack_x(v[b].T))
        maskT_b.append(np.ascontiguousarray(mask[b, 0].T).astype(BF16))
    Wq_s, Wk_s, Wv_s, Wfc_s = (
        np.asarray(w, F32) * WS for w in (Wq, Wk, Wv, Wfc)
    )
    bq_f, bk_f = np.asarray(bq, F32), np.asarray(bk, F32)
    for c in range(NCORES):
        b, g = c // TPG, c % TPG
        cols = slice(g * DC, (g + 1) * DC)
        wfc_slice = Wfc_s[cols, :]  # [256, H]
        wfcp = np.ascontiguousarray(
            wfc_slice.reshape(2, 128, H).transpose(1, 0, 2).reshape(128, 2 * H)
        ).astype(FP8)
        bqk = np.stack(
            [
                bq_f[cols][:128], bq_f[cols][128:],
                bk_f[cols][:128], bk_f[cols][128:],
            ],
            axis=1,
        )
        in_maps.append({
            "qp": qp_b[b],
            "kp": kp_b[b],
            "vp": vp_b[b],
            "maskT": maskT_b[b],
            "wqp": _pack_w(Wq_s[:, cols]),
            "wkp": _pack_w(Wk_s[:, cols]),
            "wvp": _pack_w(Wv_s[:, cols]),
            "wfcp": wfcp,
            "bqk": np.ascontiguousarray(bqk),
            "bv": np.asarray(bv, F32)[cols].reshape(1, DC),
            "resid": np.ascontiguousarray(
                np.concatenate(
                    [
                        q[b, 512 * i + 128 * g : 512 * i + 128 * (g + 1)]
                        for i in range(RT)
                    ]
                )
                + np.asarray(bfc, F32)[None, :]
            ).astype(BF16),
            "gamma": np.asarray(gamma, F32).reshape(1, H),
            "beta": np.asarray(beta, F32).reshape(1, H),
        })
    return in_maps


_LAST_RUN_S = [None]


def kernel(q, k, v, mask, Wq, bq, Wk, bk, Wv, bv, Wfc, bfc, gamma, beta):
    import time

    nc = _get_nc()
    in_maps = _prep_inputs(
        q, k, v, mask, Wq, bq, Wk, bk, Wv, bv, Wfc, bfc, gamma, beta
    )
    t0 = time.perf_counter()
    res = run_bass_kernel_spmd(nc, in_maps, list(range(NCORES)))
    _LAST_RUN_S[0] = time.perf_counter() - t0
    full = np.empty((B, S, H), F32)
    for c in range(NCORES):
        b, r = c // TPG, c % TPG
        o = np.asarray(res.results[c]["out"], dtype=F32)
        for i in range(RT):
            full[b, 512 * i + 128 * r : 512 * i + 128 * (r + 1)] = o[
                128 * i : 128 * (i + 1)
            ]
    return full
